# revision 9
# baseline (speedup 1.0000x reference)
"""Trainium2 Bass kernel for BaseCausalWanSelfAttention (local+sink sparse attention
with interleaved rotary), SPMD across 8 NeuronCores.

Sharding: the 24 (batch, head) pairs are split 3-per-core across 8 cores; each
core runs full local+sink attention for its pairs independently (no collectives).
"""
import sys

sys.path.insert(0, "/opt/trn_rl_repo")

import numpy as np

import concourse.bacc as bacc
import concourse.tile as tile
import concourse.mybir as mybir

dt = mybir.dt

# Problem config (hardcoded per contest contract)
B, S, H, D = 2, 3072, 12, 128
LOCAL_WINDOW = 1560
SINK = 128
N_CORES = 8
PER_CORE = (B * H) // N_CORES  # 3
QB = 512  # query block (columns of transposed scores)
NQC = QB // 128  # 128-query chunks per block
SCALE = 1.0 / float(np.sqrt(D))


def _window_partial_deltas(w):
    """k-tile offsets (qi - kj) where the local-window edge cuts through the
    128x128 tile; maps delta -> threshold T with allowed iff (c - p) < T."""
    out = {}
    for d in range((w - 127 + 127) // 128, (w + 127) // 128 + 1):
        t = w - 128 * d
        if -127 <= t <= 127:
            out[d] = t
    return out


def chunk_kinds(qb, kj, w=LOCAL_WINDOW, nqc=NQC):
    """Per 128-query chunk classification of k-tile kj for query block qb.
    Returns list of (t, kind) with kind in {"full", "diag", ("win", delta)} for
    valid chunks only. SINK==128 assumed (k-tile 0 fully attendable)."""
    partial = _window_partial_deltas(w)
    max_delta = max(partial) if partial else (w - 1) // 128
    kinds = []
    for t in range(nqc):
        qi = nqc * qb + t
        if kj == 0:
            kinds.append((t, "diag" if qi == 0 else "full"))
            continue
        delta = qi - kj
        if delta < 0 or delta > max_delta:
            continue
        if delta == 0:
            kinds.append((t, "diag"))
        elif delta in partial:
            kinds.append((t, ("win", delta)))
        else:
            kinds.append((t, "full"))
    return kinds


def kj_list(qb, s=S, w=LOCAL_WINDOW, nqc=NQC):
    partial = _window_partial_deltas(w)
    max_delta = max(partial) if partial else (w - 1) // 128
    n_ktiles = s // 128
    hi = min(nqc * qb + nqc - 1, n_ktiles - 1)
    lo = max(1, nqc * qb - max_delta)
    return [0] + [kj for kj in range(lo, hi + 1)]


def build_nc(s=S, per_core=PER_CORE, w=LOCAL_WINDOW):
    """Build the SPMD single-core program (identical on all cores)."""
    nqb = s // QB
    partial = _window_partial_deltas(w)

    nc = bacc.Bacc("TRN2", target_bir_lowering=False, debug=False)

    qT = nc.declare_dram_parameter("qT", [per_core, 128, s], dt.float32r, isOutput=False)
    kT = nc.declare_dram_parameter("kT", [per_core, 128, s], dt.float32r, isOutput=False)
    v = nc.declare_dram_parameter("v", [per_core, s, 128], dt.float32r, isOutput=False)
    cexpT = nc.declare_dram_parameter("cexpT", [128, s], dt.float32r, isOutput=False)
    ssigT = nc.declare_dram_parameter("ssigT", [128, s], dt.float32r, isOutput=False)
    pswap = nc.declare_dram_parameter("pswap", [128, 128], dt.float32r, isOutput=False)
    ident = nc.declare_dram_parameter("ident", [128, 128], dt.float32, isOutput=False)
    ones1 = nc.declare_dram_parameter("ones1", [128, 1], dt.float32r, isOutput=False)
    maskD = nc.declare_dram_parameter("maskD", [128, 128], dt.float32r, isOutput=False)
    wmask_names = {}
    for delta in sorted(partial):
        nm = f"maskW{delta}"
        wmask_names[delta] = nc.declare_dram_parameter(
            nm, [128, 128], dt.float32r, isOutput=False
        )
    out = nc.declare_dram_parameter("out", [per_core, s, 128], dt.float32, isOutput=True)

    with tile.TileContext(nc) as tc:
        with (
            tc.tile_pool(name="const", bufs=1) as cpool,
            tc.tile_pool(name="big", bufs=2) as bigpool,
            tc.tile_pool(name="probs", bufs=4) as ppool,
            tc.tile_pool(name="tail", bufs=2) as tpool,
            tc.tile_pool(name="ps_sc", bufs=2, space="PSUM") as ps_sc,
            tc.tile_pool(name="ps_out", bufs=2, space="PSUM") as ps_out,
            tc.tile_pool(name="ps_den", bufs=1, space="PSUM") as ps_den,
            tc.tile_pool(name="ps_tr", bufs=1, space="PSUM") as ps_tr,
        ):
            # constants
            cexp_sb = cpool.tile([128, s], dt.float32r, tag="cexp")
            ssig_sb = cpool.tile([128, s], dt.float32r, tag="ssig")
            nc.sync.dma_start(out=cexp_sb[:], in_=cexpT[:])
            nc.sync.dma_start(out=ssig_sb[:], in_=ssigT[:])
            pswap_sb = cpool.tile([128, 128], dt.float32r, tag="pswap")
            ident_sb = cpool.tile([128, 128], dt.float32, tag="ident")
            ones1_sb = cpool.tile([128, 1], dt.float32r, tag="ones1")
            nc.sync.dma_start(out=pswap_sb[:], in_=pswap[:])
            nc.sync.dma_start(out=ident_sb[:], in_=ident[:])
            nc.sync.dma_start(out=ones1_sb[:], in_=ones1[:])
            maskD_sb = cpool.tile([128, 128], dt.float32r, tag="maskD")
            nc.sync.dma_start(out=maskD_sb[:], in_=maskD[:])
            wmask_sb = {}
            wdeltas = sorted(wmask_names)
            for delta, ap in wmask_names.items():
                m = cpool.tile([128, 128], dt.float32r, tag=f"maskW{delta}")
                nc.sync.dma_start(out=m[:], in_=ap[:])
                wmask_sb[delta] = m
            # combined [W_d | W_{d+1}] tile for adjacent window chunks
            wpair_sb = None
            if len(wdeltas) == 2 and wdeltas[1] == wdeltas[0] + 1:
                wpair_sb = cpool.tile([128, 256], dt.float32r, tag="maskWpair")
                nc.sync.dma_start(
                    out=wpair_sb[:, 0:128], in_=wmask_names[wdeltas[0]][:]
                )
                nc.sync.dma_start(
                    out=wpair_sb[:, 128:256], in_=wmask_names[wdeltas[1]][:]
                )

            def prep(u):
                """Load + rotary for unit u; returns (rq, rk, v_sb)."""
                qraw = bigpool.tile([128, s], dt.float32r, tag="qraw", bufs=1)
                kraw = bigpool.tile([128, s], dt.float32r, tag="kraw", bufs=1)
                v_sb = bigpool.tile([128, s], dt.float32r, tag="v")
                nc.sync.dma_start(out=qraw[:], in_=qT[u])
                nc.sync.dma_start(out=kraw[:], in_=kT[u])
                nc.sync.dma_start(
                    out=v_sb[:].rearrange("p (n d) -> p n d", d=128),
                    in_=v[u].rearrange("(n p) d -> p n d", p=128),
                )
                rots = {}
                for name, raw in (("q", qraw), ("k", kraw)):
                    r = bigpool.tile([128, s], dt.float32r, tag=f"r{name}")
                    swaps = []
                    for c2 in range(s // 1024):
                        sw = ps_sc.tile([128, 1024], dt.float32, tag="sc")
                        for hh in range(2):
                            c = c2 * 2 + hh
                            sl = slice(c * 512, (c + 1) * 512)
                            nc.tensor.matmul(
                                sw[:, hh * 512 : (hh + 1) * 512],
                                pswap_sb[:],
                                raw[:, sl],
                                start=True,
                                stop=True,
                            )
                        swaps.append((c2, sw))
                    # r = raw * cexp  (raw fully consumed after this)
                    nc.vector.tensor_mul(r[:], raw[:], cexp_sb[:])
                    # raw <- swap(raw) * ssig   (reuse raw as scratch)
                    for c2, sw in swaps:
                        sl = slice(c2 * 1024, (c2 + 1) * 1024)
                        nc.vector.tensor_mul(
                            raw[:, sl], sw[:].bitcast(dt.float32r), ssig_sb[:, sl]
                        )
                    # r += scratch  (gpsimd; sbuf-only operands)
                    nc.gpsimd.tensor_add(r[:], r[:], raw[:])
                    rots[name] = r
                return rots["q"], rots["k"], v_sb

            def attention(u, rq, rk, v_sb):
                for qb in range(nqb):
                    outT_ps = ps_out.tile([128, QB], dt.float32, tag="outT")
                    den_ps = ps_den.tile([1, QB], dt.float32, tag="den")
                    kjs = kj_list(qb, s=s, w=w)
                    tiles = []
                    for kj in kjs:
                        kinds = chunk_kinds(qb, kj, w=w)
                        assert kinds, (qb, kj)
                        tiles.append((kj, kinds, kinds[0][0], kinds[-1][0] + 1))
                    # process full-width tiles first (kj=0 leads; pairs share
                    # one 2-bank psum tile + one exp), then partial tiles
                    fulls = [x for x in tiles if x[3] - x[2] == NQC]
                    parts = [x for x in tiles if x[3] - x[2] != NQC]
                    assert fulls[0][0] == 0
                    groups = []
                    i = 0
                    while i < len(fulls):
                        groups.append(fulls[i : i + 2])
                        i += 2
                    groups.extend([x] for x in parts)
                    n_groups = len(groups)

                    for gi, group in enumerate(groups):
                        lastg = gi == n_groups - 1
                        sc = ps_sc.tile([128, 2 * QB], dt.float32, tag="sc")
                        probs = ppool.tile([128, 2 * QB], dt.float32r, tag="probs")
                        for hh, (kj, kinds, t0, t1) in enumerate(group):
                            csl = slice(qb * QB + t0 * 128, qb * QB + t1 * 128)
                            hsl = slice(hh * QB + t0 * 128, hh * QB + t1 * 128)
                            ksl = slice(kj * 128, (kj + 1) * 128)
                            nc.tensor.matmul(
                                sc[:, hsl], rk[:, ksl], rq[:, csl],
                                start=True, stop=True,
                            )
                        lo = group[0][2] * 128
                        hi = (len(group) - 1) * QB + group[-1][3] * 128
                        nc.scalar.activation(
                            probs[:, lo:hi],
                            sc[:, lo:hi],
                            mybir.ActivationFunctionType.Exp,
                            scale=SCALE,
                        )
                        for hh, (kj, kinds, t0, t1) in enumerate(group):
                            # coalesce adjacent window masks into one op
                            mk = [k for k in kinds if k[1] != "full"]
                            j = 0
                            while j < len(mk):
                                t, kind = mk[j]
                                if (
                                    wpair_sb is not None
                                    and j + 1 < len(mk)
                                    and kind != "diag"
                                    and mk[j + 1][1] != "diag"
                                    and mk[j + 1][0] == t + 1
                                    and kind[1] == wdeltas[0]
                                ):
                                    tsl = slice(
                                        hh * QB + t * 128, hh * QB + (t + 2) * 128
                                    )
                                    nc.vector.tensor_mul(
                                        probs[:, tsl], probs[:, tsl], wpair_sb[:]
                                    )
                                    j += 2
                                    continue
                                m = (
                                    maskD_sb
                                    if kind == "diag"
                                    else wmask_sb[kind[1]]
                                )
                                tsl = slice(
                                    hh * QB + t * 128, hh * QB + (t + 1) * 128
                                )
                                nc.vector.tensor_mul(
                                    probs[:, tsl], probs[:, tsl], m[:]
                                )
                                j += 1
                            psl = slice(t0 * 128, t1 * 128)
                            hsl = slice(hh * QB + t0 * 128, hh * QB + t1 * 128)
                            ksl = slice(kj * 128, (kj + 1) * 128)
                            first = kj == 0
                            last = lastg and hh == len(group) - 1
                            nc.tensor.matmul(
                                outT_ps[:, psl],
                                v_sb[:, ksl],
                                probs[:, hsl],
                                start=first,
                                stop=last,
                            )
                            nc.tensor.matmul(
                                den_ps[:, psl],
                                ones1_sb[:],
                                probs[:, hsl],
                                start=first,
                                stop=last,
                            )

                    # ---- tail: normalize, transpose, store ----
                    rden = tpool.tile([1, QB], dt.float32, tag="rden")
                    nc.vector.reciprocal_approx_fast(rden[:], den_ps[:])
                    rdenb = tpool.tile([128, QB], dt.float32, tag="rdenb")
                    nc.gpsimd.partition_broadcast(rdenb[:], rden[:])
                    outN = tpool.tile([128, QB], dt.float32, tag="outN")
                    nc.vector.tensor_mul(outN[:], outT_ps[:], rdenb[:])
                    tr = ps_tr.tile([128, QB], dt.float32, tag="tr")
                    for c in range(NQC):
                        tsl = slice(c * 128, (c + 1) * 128)
                        nc.tensor.transpose(tr[:, tsl], outN[:, tsl], ident_sb[:])
                    out_sb = tpool.tile([128, QB], dt.float32, tag="out_sb")
                    nc.scalar.copy(out_sb[:], tr[:])
                    nc.sync.dma_start(
                        out=out[u].rearrange("(n p) d -> p n d", p=128)[
                            :, qb * NQC : (qb + 1) * NQC, :
                        ],
                        in_=out_sb[:].rearrange("p (n d) -> p n d", d=128),
                    )

            pending = [prep(0)]
            for u in range(per_core):
                if u + 1 < per_core:
                    pending.append(prep(u + 1))
                attention(u, *pending.pop(0))

    nc.compile()
    return nc


def host_prep(q, k, v, cos, sin, s=S, w=LOCAL_WINDOW):
    """Build per-core input maps from full inputs."""
    b, _, h, d = q.shape
    partial = _window_partial_deltas(w)

    cexp = np.empty((128, s), dtype=np.float32)
    ssig = np.empty((128, s), dtype=np.float32)
    cexp[0::2, :] = cos.T
    cexp[1::2, :] = cos.T
    ssig[0::2, :] = -sin.T
    ssig[1::2, :] = sin.T

    pswap = np.zeros((128, 128), dtype=np.float32)
    idx = np.arange(128)
    pswap[idx, idx ^ 1] = 1.0
    ident = np.eye(128, dtype=np.float32)
    ones1 = np.ones((128, 1), dtype=np.float32)

    p = np.arange(128)[:, None]
    c = np.arange(128)[None, :]
    maskD = (c >= p).astype(np.float32)
    wmasks = {
        delta: ((c - p) < t).astype(np.float32) for delta, t in partial.items()
    }

    units = [(bi, hi) for bi in range(b) for hi in range(h)]
    per = len(units) // N_CORES
    in_maps = []
    for core in range(N_CORES):
        us = units[core * per : (core + 1) * per]
        qTc = np.ascontiguousarray(
            np.stack([q[bi, :, hi, :].T for bi, hi in us])
        )
        kTc = np.ascontiguousarray(
            np.stack([k[bi, :, hi, :].T for bi, hi in us])
        )
        vc = np.ascontiguousarray(np.stack([v[bi, :, hi, :] for bi, hi in us]))
        m = {
            "qT": qTc,
            "kT": kTc,
            "v": vc,
            "cexpT": cexp,
            "ssigT": ssig,
            "pswap": pswap,
            "ident": ident,
            "ones1": ones1,
            "maskD": maskD,
        }
        for delta, msk in wmasks.items():
            m[f"maskW{delta}"] = msk
        in_maps.append(m)
    return in_maps, units


_NC_CACHE = {}


def kernel(q, k, v, cos, sin):
    from concourse.bass_utils import run_bass_kernel_spmd

    q = np.asarray(q, dtype=np.float32)
    k = np.asarray(k, dtype=np.float32)
    v = np.asarray(v, dtype=np.float32)
    cos = np.asarray(cos, dtype=np.float32)
    sin = np.asarray(sin, dtype=np.float32)

    if "nc" not in _NC_CACHE:
        _NC_CACHE["nc"] = build_nc()
    nc = _NC_CACHE["nc"]

    in_maps, units = host_prep(q, k, v, cos, sin)
    res = run_bass_kernel_spmd(nc, in_maps, core_ids=list(range(N_CORES)))

    b, s, h, d = q.shape
    full = np.empty((b, s, h, d), dtype=np.float32)
    per = len(units) // N_CORES
    for core in range(N_CORES):
        o = res.results[core]["out"]  # [per, s, 128]
        for i, (bi, hi) in enumerate(units[core * per : (core + 1) * per]):
            full[bi, :, hi, :] = o[i]
    return full


# revision 11
# speedup vs baseline: 1.0814x; 1.0814x over previous
"""Trainium2 Bass kernel for BaseCausalWanSelfAttention (local+sink sparse attention
with interleaved rotary), SPMD across 8 NeuronCores.

Sharding: the 24 (batch, head) pairs are split 3-per-core across 8 cores; each
core runs full local+sink attention for its pairs independently (no collectives).
"""
import sys

sys.path.insert(0, "/opt/trn_rl_repo")

import numpy as np

import concourse.bacc as bacc
import concourse.tile as tile
import concourse.mybir as mybir

dt = mybir.dt

# Problem config (hardcoded per contest contract)
B, S, H, D = 2, 3072, 12, 128
LOCAL_WINDOW = 1560
SINK = 128
N_CORES = 8
PER_CORE = (B * H) // N_CORES  # 3
QB = 512  # query block (columns of transposed scores)
NQC = QB // 128  # 128-query chunks per block
SCALE = 1.0 / float(np.sqrt(D))


def _window_partial_deltas(w):
    """k-tile offsets (qi - kj) where the local-window edge cuts through the
    128x128 tile; maps delta -> threshold T with allowed iff (c - p) < T."""
    out = {}
    for d in range((w - 127 + 127) // 128, (w + 127) // 128 + 1):
        t = w - 128 * d
        if -127 <= t <= 127:
            out[d] = t
    return out


def chunk_kinds(qb, kj, w=LOCAL_WINDOW, nqc=NQC):
    """Per 128-query chunk classification of k-tile kj for query block qb.
    Returns list of (t, kind) with kind in {"full", "diag", ("win", delta)} for
    valid chunks only. SINK==128 assumed (k-tile 0 fully attendable)."""
    partial = _window_partial_deltas(w)
    max_delta = max(partial) if partial else (w - 1) // 128
    kinds = []
    for t in range(nqc):
        qi = nqc * qb + t
        if kj == 0:
            kinds.append((t, "diag" if qi == 0 else "full"))
            continue
        delta = qi - kj
        if delta < 0 or delta > max_delta:
            continue
        if delta == 0:
            kinds.append((t, "diag"))
        elif delta in partial:
            kinds.append((t, ("win", delta)))
        else:
            kinds.append((t, "full"))
    return kinds


def kj_list(qb, s=S, w=LOCAL_WINDOW, nqc=NQC):
    partial = _window_partial_deltas(w)
    max_delta = max(partial) if partial else (w - 1) // 128
    n_ktiles = s // 128
    hi = min(nqc * qb + nqc - 1, n_ktiles - 1)
    lo = max(1, nqc * qb - max_delta)
    return [0] + [kj for kj in range(lo, hi + 1)]


def build_nc(s=S, per_core=PER_CORE, w=LOCAL_WINDOW):
    """Build the SPMD single-core program (identical on all cores)."""
    nqb = s // QB
    partial = _window_partial_deltas(w)

    nc = bacc.Bacc("TRN2", target_bir_lowering=False, debug=False)

    qT = nc.declare_dram_parameter("qT", [per_core, 128, s], dt.float32r, isOutput=False)
    kT = nc.declare_dram_parameter("kT", [per_core, 128, s], dt.float32r, isOutput=False)
    v = nc.declare_dram_parameter("v", [per_core, s, 128], dt.float32r, isOutput=False)
    cexpT = nc.declare_dram_parameter("cexpT", [128, s], dt.float32r, isOutput=False)
    ssigT = nc.declare_dram_parameter("ssigT", [128, s], dt.float32r, isOutput=False)
    pswap = nc.declare_dram_parameter("pswap", [128, 128], dt.float32r, isOutput=False)
    ident = nc.declare_dram_parameter("ident", [128, 128], dt.float32, isOutput=False)
    ones = nc.declare_dram_parameter("ones", [128, 128], dt.float32r, isOutput=False)
    maskD = nc.declare_dram_parameter("maskD", [128, 128], dt.float32r, isOutput=False)
    wmask_names = {}
    for delta in sorted(partial):
        nm = f"maskW{delta}"
        wmask_names[delta] = nc.declare_dram_parameter(
            nm, [128, 128], dt.float32r, isOutput=False
        )
    out = nc.declare_dram_parameter("out", [per_core, s, 128], dt.float32, isOutput=True)

    with tile.TileContext(nc) as tc:
        with (
            tc.tile_pool(name="const", bufs=1) as cpool,
            tc.tile_pool(name="big", bufs=2) as bigpool,
            tc.tile_pool(name="probs", bufs=4) as ppool,
            tc.tile_pool(name="tail", bufs=2) as tpool,
            tc.tile_pool(name="ps_sc", bufs=2, space="PSUM") as ps_sc,
            tc.tile_pool(name="ps_out", bufs=2, space="PSUM") as ps_out,
            tc.tile_pool(name="ps_den", bufs=1, space="PSUM") as ps_den,
            tc.tile_pool(name="ps_tr", bufs=1, space="PSUM") as ps_tr,
        ):
            # constants
            cexp_sb = cpool.tile([128, s], dt.float32r, tag="cexp")
            ssig_sb = cpool.tile([128, s], dt.float32r, tag="ssig")
            nc.sync.dma_start(out=cexp_sb[:], in_=cexpT[:])
            nc.sync.dma_start(out=ssig_sb[:], in_=ssigT[:])
            pswap_sb = cpool.tile([128, 128], dt.float32r, tag="pswap")
            ident_sb = cpool.tile([128, 128], dt.float32, tag="ident")
            ones_sb = cpool.tile([128, 128], dt.float32r, tag="ones")
            nc.sync.dma_start(out=pswap_sb[:], in_=pswap[:])
            nc.sync.dma_start(out=ident_sb[:], in_=ident[:])
            nc.sync.dma_start(out=ones_sb[:], in_=ones[:])
            maskD_sb = cpool.tile([128, 128], dt.float32r, tag="maskD")
            nc.sync.dma_start(out=maskD_sb[:], in_=maskD[:])
            wmask_sb = {}
            wdeltas = sorted(wmask_names)
            for delta, ap in wmask_names.items():
                m = cpool.tile([128, 128], dt.float32r, tag=f"maskW{delta}")
                nc.sync.dma_start(out=m[:], in_=ap[:])
                wmask_sb[delta] = m
            # combined [W_d | W_{d+1}] tile for adjacent window chunks
            wpair_sb = None
            if len(wdeltas) == 2 and wdeltas[1] == wdeltas[0] + 1:
                wpair_sb = cpool.tile([128, 256], dt.float32r, tag="maskWpair")
                nc.sync.dma_start(
                    out=wpair_sb[:, 0:128], in_=wmask_names[wdeltas[0]][:]
                )
                nc.sync.dma_start(
                    out=wpair_sb[:, 128:256], in_=wmask_names[wdeltas[1]][:]
                )

            def prep(u):
                """Load + rotary for unit u; returns (rq, rk, v_sb)."""
                qraw = bigpool.tile([128, s], dt.float32r, tag="qraw", bufs=1)
                kraw = bigpool.tile([128, s], dt.float32r, tag="kraw", bufs=1)
                v_sb = bigpool.tile([128, s], dt.float32r, tag="v")
                nc.sync.dma_start(out=qraw[:], in_=qT[u])
                nc.sync.dma_start(out=kraw[:], in_=kT[u])
                nc.sync.dma_start(
                    out=v_sb[:].rearrange("p (n d) -> p n d", d=128),
                    in_=v[u].rearrange("(n p) d -> p n d", p=128),
                )
                rots = {}
                for name, raw in (("q", qraw), ("k", kraw)):
                    r = bigpool.tile([128, s], dt.float32r, tag=f"r{name}")
                    swaps = []
                    for c2 in range(s // 1024):
                        sw = ps_sc.tile([128, 1024], dt.float32, tag="sc")
                        for hh in range(2):
                            c = c2 * 2 + hh
                            sl = slice(c * 512, (c + 1) * 512)
                            nc.tensor.matmul(
                                sw[:, hh * 512 : (hh + 1) * 512],
                                pswap_sb[:],
                                raw[:, sl],
                                start=True,
                                stop=True,
                            )
                        swaps.append((c2, sw))
                    # r = raw * cexp  (gpsimd; raw fully consumed after this)
                    nc.gpsimd.tensor_mul(r[:], raw[:], cexp_sb[:])
                    # raw <- swap(raw) * ssig  (dve, 512-wide chunks; psum src)
                    for c2, sw in swaps:
                        for hh in range(2):
                            c = c2 * 2 + hh
                            sl = slice(c * 512, (c + 1) * 512)
                            nc.vector.tensor_mul(
                                raw[:, sl],
                                sw[:, hh * 512 : (hh + 1) * 512].bitcast(dt.float32r),
                                ssig_sb[:, sl],
                            )
                    # r += scratch  (gpsimd; sbuf-only operands)
                    nc.gpsimd.tensor_add(r[:], r[:], raw[:])
                    rots[name] = r
                return rots["q"], rots["k"], v_sb

            def attention(u, rq, rk, v_sb):
                for qb in range(nqb):
                    outT_ps = ps_out.tile([128, QB], dt.float32, tag="outT")
                    den_ps = ps_den.tile([128, QB], dt.float32, tag="den")
                    kjs = kj_list(qb, s=s, w=w)
                    tiles = []
                    for kj in kjs:
                        kinds = chunk_kinds(qb, kj, w=w)
                        assert kinds, (qb, kj)
                        tiles.append((kj, kinds, kinds[0][0], kinds[-1][0] + 1))
                    # process full-width tiles first (kj=0 leads; pairs share
                    # one 2-bank psum tile + one exp), then partial tiles
                    fulls = [x for x in tiles if x[3] - x[2] == NQC]
                    parts = [x for x in tiles if x[3] - x[2] != NQC]
                    assert fulls[0][0] == 0
                    groups = []
                    i = 0
                    while i < len(fulls):
                        groups.append(fulls[i : i + 2])
                        i += 2
                    groups.extend([x] for x in parts)
                    n_groups = len(groups)

                    for gi, group in enumerate(groups):
                        lastg = gi == n_groups - 1
                        sc = ps_sc.tile([128, 2 * QB], dt.float32, tag="sc")
                        probs = ppool.tile([128, 2 * QB], dt.float32r, tag="probs")
                        for hh, (kj, kinds, t0, t1) in enumerate(group):
                            csl = slice(qb * QB + t0 * 128, qb * QB + t1 * 128)
                            hsl = slice(hh * QB + t0 * 128, hh * QB + t1 * 128)
                            ksl = slice(kj * 128, (kj + 1) * 128)
                            nc.tensor.matmul(
                                sc[:, hsl], rk[:, ksl], rq[:, csl],
                                start=True, stop=True,
                            )
                        lo = group[0][2] * 128
                        hi = (len(group) - 1) * QB + group[-1][3] * 128
                        nc.scalar.activation(
                            probs[:, lo:hi],
                            sc[:, lo:hi],
                            mybir.ActivationFunctionType.Exp,
                            scale=SCALE,
                        )
                        for hh, (kj, kinds, t0, t1) in enumerate(group):
                            # coalesce adjacent window masks into one op
                            mk = [k for k in kinds if k[1] != "full"]
                            j = 0
                            while j < len(mk):
                                t, kind = mk[j]
                                if (
                                    wpair_sb is not None
                                    and j + 1 < len(mk)
                                    and kind != "diag"
                                    and mk[j + 1][1] != "diag"
                                    and mk[j + 1][0] == t + 1
                                    and kind[1] == wdeltas[0]
                                ):
                                    tsl = slice(
                                        hh * QB + t * 128, hh * QB + (t + 2) * 128
                                    )
                                    nc.vector.tensor_mul(
                                        probs[:, tsl], probs[:, tsl], wpair_sb[:]
                                    )
                                    j += 2
                                    continue
                                m = (
                                    maskD_sb
                                    if kind == "diag"
                                    else wmask_sb[kind[1]]
                                )
                                tsl = slice(
                                    hh * QB + t * 128, hh * QB + (t + 1) * 128
                                )
                                nc.vector.tensor_mul(
                                    probs[:, tsl], probs[:, tsl], m[:]
                                )
                                j += 1
                            psl = slice(t0 * 128, t1 * 128)
                            hsl = slice(hh * QB + t0 * 128, hh * QB + t1 * 128)
                            ksl = slice(kj * 128, (kj + 1) * 128)
                            first = kj == 0
                            last = lastg and hh == len(group) - 1
                            nc.tensor.matmul(
                                outT_ps[:, psl],
                                v_sb[:, ksl],
                                probs[:, hsl],
                                start=first,
                                stop=last,
                            )
                            nc.tensor.matmul(
                                den_ps[:, psl],
                                ones_sb[:],
                                probs[:, hsl],
                                start=first,
                                stop=last,
                            )

                    # ---- tail: normalize, transpose, store ----
                    rden = tpool.tile([128, QB], dt.float32, tag="rden")
                    nc.vector.reciprocal_approx_fast(rden[:], den_ps[:])
                    outN = tpool.tile([128, QB], dt.float32, tag="outN")
                    nc.vector.tensor_mul(outN[:], outT_ps[:], rden[:])
                    tr = ps_tr.tile([128, QB], dt.float32, tag="tr")
                    for c in range(NQC):
                        tsl = slice(c * 128, (c + 1) * 128)
                        nc.tensor.transpose(tr[:, tsl], outN[:, tsl], ident_sb[:])
                    out_sb = tpool.tile([128, QB], dt.float32, tag="out_sb")
                    nc.scalar.copy(out_sb[:], tr[:])
                    nc.sync.dma_start(
                        out=out[u].rearrange("(n p) d -> p n d", p=128)[
                            :, qb * NQC : (qb + 1) * NQC, :
                        ],
                        in_=out_sb[:].rearrange("p (n d) -> p n d", d=128),
                    )

            pending = [prep(0)]
            for u in range(per_core):
                if u + 1 < per_core:
                    pending.append(prep(u + 1))
                attention(u, *pending.pop(0))

    nc.compile()
    return nc


def host_prep(q, k, v, cos, sin, s=S, w=LOCAL_WINDOW):
    """Build per-core input maps from full inputs."""
    b, _, h, d = q.shape
    partial = _window_partial_deltas(w)

    cexp = np.empty((128, s), dtype=np.float32)
    ssig = np.empty((128, s), dtype=np.float32)
    cexp[0::2, :] = cos.T
    cexp[1::2, :] = cos.T
    ssig[0::2, :] = -sin.T
    ssig[1::2, :] = sin.T

    pswap = np.zeros((128, 128), dtype=np.float32)
    idx = np.arange(128)
    pswap[idx, idx ^ 1] = 1.0
    ident = np.eye(128, dtype=np.float32)
    ones = np.ones((128, 128), dtype=np.float32)

    p = np.arange(128)[:, None]
    c = np.arange(128)[None, :]
    maskD = (c >= p).astype(np.float32)
    wmasks = {
        delta: ((c - p) < t).astype(np.float32) for delta, t in partial.items()
    }

    units = [(bi, hi) for bi in range(b) for hi in range(h)]
    per = len(units) // N_CORES
    in_maps = []
    for core in range(N_CORES):
        us = units[core * per : (core + 1) * per]
        qTc = np.ascontiguousarray(
            np.stack([q[bi, :, hi, :].T for bi, hi in us])
        )
        kTc = np.ascontiguousarray(
            np.stack([k[bi, :, hi, :].T for bi, hi in us])
        )
        vc = np.ascontiguousarray(np.stack([v[bi, :, hi, :] for bi, hi in us]))
        m = {
            "qT": qTc,
            "kT": kTc,
            "v": vc,
            "cexpT": cexp,
            "ssigT": ssig,
            "pswap": pswap,
            "ident": ident,
            "ones": ones,
            "maskD": maskD,
        }
        for delta, msk in wmasks.items():
            m[f"maskW{delta}"] = msk
        in_maps.append(m)
    return in_maps, units


_NC_CACHE = {}


def kernel(q, k, v, cos, sin):
    from concourse.bass_utils import run_bass_kernel_spmd

    q = np.asarray(q, dtype=np.float32)
    k = np.asarray(k, dtype=np.float32)
    v = np.asarray(v, dtype=np.float32)
    cos = np.asarray(cos, dtype=np.float32)
    sin = np.asarray(sin, dtype=np.float32)

    if "nc" not in _NC_CACHE:
        _NC_CACHE["nc"] = build_nc()
    nc = _NC_CACHE["nc"]

    in_maps, units = host_prep(q, k, v, cos, sin)
    res = run_bass_kernel_spmd(nc, in_maps, core_ids=list(range(N_CORES)))

    b, s, h, d = q.shape
    full = np.empty((b, s, h, d), dtype=np.float32)
    per = len(units) // N_CORES
    for core in range(N_CORES):
        o = res.results[core]["out"]  # [per, s, 128]
        for i, (bi, hi) in enumerate(units[core * per : (core + 1) * per]):
            full[bi, :, hi, :] = o[i]
    return full


# revision 13
# speedup vs baseline: 1.1697x; 1.0816x over previous
"""Trainium2 Bass kernel for BaseCausalWanSelfAttention (local+sink sparse attention
with interleaved rotary), SPMD across 8 NeuronCores.

Sharding: the 24 (batch, head) pairs are split 3-per-core across 8 cores; each
core runs full local+sink attention for its pairs independently (no collectives).
"""
import sys

sys.path.insert(0, "/opt/trn_rl_repo")

import numpy as np

import concourse.bacc as bacc
import concourse.tile as tile
import concourse.mybir as mybir

dt = mybir.dt

# Problem config (hardcoded per contest contract)
B, S, H, D = 2, 3072, 12, 128
LOCAL_WINDOW = 1560
SINK = 128
N_CORES = 8
PER_CORE = (B * H) // N_CORES  # 3
QB = 512  # query block (columns of transposed scores)
NQC = QB // 128  # 128-query chunks per block
SCALE = 1.0 / float(np.sqrt(D))


def _window_partial_deltas(w):
    """k-tile offsets (qi - kj) where the local-window edge cuts through the
    128x128 tile; maps delta -> threshold T with allowed iff (c - p) < T."""
    out = {}
    for d in range((w - 127 + 127) // 128, (w + 127) // 128 + 1):
        t = w - 128 * d
        if -127 <= t <= 127:
            out[d] = t
    return out


def chunk_kinds(qb, kj, w=LOCAL_WINDOW, nqc=NQC):
    """Per 128-query chunk classification of k-tile kj for query block qb.
    Returns list of (t, kind) with kind in {"full", "diag", ("win", delta)} for
    valid chunks only. SINK==128 assumed (k-tile 0 fully attendable)."""
    partial = _window_partial_deltas(w)
    max_delta = max(partial) if partial else (w - 1) // 128
    kinds = []
    for t in range(nqc):
        qi = nqc * qb + t
        if kj == 0:
            kinds.append((t, "diag" if qi == 0 else "full"))
            continue
        delta = qi - kj
        if delta < 0 or delta > max_delta:
            continue
        if delta == 0:
            kinds.append((t, "diag"))
        elif delta in partial:
            kinds.append((t, ("win", delta)))
        else:
            kinds.append((t, "full"))
    return kinds


def kj_list(qb, s=S, w=LOCAL_WINDOW, nqc=NQC):
    partial = _window_partial_deltas(w)
    max_delta = max(partial) if partial else (w - 1) // 128
    n_ktiles = s // 128
    hi = min(nqc * qb + nqc - 1, n_ktiles - 1)
    lo = max(1, nqc * qb - max_delta)
    return [0] + [kj for kj in range(lo, hi + 1)]


def build_nc(s=S, per_core=PER_CORE, w=LOCAL_WINDOW):
    """Build the SPMD single-core program (identical on all cores)."""
    nqb = s // QB
    partial = _window_partial_deltas(w)

    nc = bacc.Bacc("TRN2", target_bir_lowering=False, debug=False)

    qT = nc.declare_dram_parameter("qT", [per_core, 128, s], dt.float32r, isOutput=False)
    kT = nc.declare_dram_parameter("kT", [per_core, 128, s], dt.float32r, isOutput=False)
    v = nc.declare_dram_parameter("v", [per_core, s, 128], dt.float32r, isOutput=False)
    cexpT = nc.declare_dram_parameter("cexpT", [128, s], dt.float32r, isOutput=False)
    ssigT = nc.declare_dram_parameter("ssigT", [128, s], dt.float32r, isOutput=False)
    pswap = nc.declare_dram_parameter("pswap", [128, 128], dt.float32r, isOutput=False)
    ident = nc.declare_dram_parameter("ident", [128, 128], dt.float32, isOutput=False)
    ones = nc.declare_dram_parameter("ones", [128, 128], dt.float32r, isOutput=False)
    maskD = nc.declare_dram_parameter("maskD", [128, 128], dt.float32r, isOutput=False)
    wmask_names = {}
    for delta in sorted(partial):
        nm = f"maskW{delta}"
        wmask_names[delta] = nc.declare_dram_parameter(
            nm, [128, 128], dt.float32r, isOutput=False
        )
    out = nc.declare_dram_parameter("out", [per_core, s, 128], dt.float32, isOutput=True)

    with tile.TileContext(nc) as tc:
        with (
            tc.tile_pool(name="const", bufs=1) as cpool,
            tc.tile_pool(name="big", bufs=2) as bigpool,
            tc.tile_pool(name="probs", bufs=4) as ppool,
            tc.tile_pool(name="tail", bufs=2) as tpool,
            tc.tile_pool(name="ps_sc", bufs=2, space="PSUM") as ps_sc,
            tc.tile_pool(name="ps_out", bufs=2, space="PSUM") as ps_out,
            tc.tile_pool(name="ps_den", bufs=1, space="PSUM") as ps_den,
            tc.tile_pool(name="ps_tr", bufs=1, space="PSUM") as ps_tr,
        ):
            # constants
            cexp_sb = cpool.tile([128, s], dt.float32r, tag="cexp")
            ssig_sb = cpool.tile([128, s], dt.float32r, tag="ssig")
            nc.sync.dma_start(out=cexp_sb[:], in_=cexpT[:])
            nc.sync.dma_start(out=ssig_sb[:], in_=ssigT[:])
            pswap_sb = cpool.tile([128, 128], dt.float32r, tag="pswap")
            ident_sb = cpool.tile([128, 128], dt.float32, tag="ident")
            ones_sb = cpool.tile([128, 128], dt.float32r, tag="ones")
            nc.sync.dma_start(out=pswap_sb[:], in_=pswap[:])
            nc.sync.dma_start(out=ident_sb[:], in_=ident[:])
            nc.sync.dma_start(out=ones_sb[:], in_=ones[:])
            maskD_sb = cpool.tile([128, 128], dt.float32r, tag="maskD")
            nc.sync.dma_start(out=maskD_sb[:], in_=maskD[:])
            wmask_sb = {}
            wdeltas = sorted(wmask_names)
            for delta, ap in wmask_names.items():
                m = cpool.tile([128, 128], dt.float32r, tag=f"maskW{delta}")
                nc.sync.dma_start(out=m[:], in_=ap[:])
                wmask_sb[delta] = m
            # combined [W_d | W_{d+1}] tile for adjacent window chunks
            wpair_sb = None
            if len(wdeltas) == 2 and wdeltas[1] == wdeltas[0] + 1:
                wpair_sb = cpool.tile([128, 256], dt.float32r, tag="maskWpair")
                nc.sync.dma_start(
                    out=wpair_sb[:, 0:128], in_=wmask_names[wdeltas[0]][:]
                )
                nc.sync.dma_start(
                    out=wpair_sb[:, 128:256], in_=wmask_names[wdeltas[1]][:]
                )

            def load(u):
                qraw = bigpool.tile([128, s], dt.float32r, tag="qraw", bufs=1)
                kraw = bigpool.tile([128, s], dt.float32r, tag="kraw", bufs=1)
                v_sb = bigpool.tile([128, s], dt.float32r, tag="v")
                nc.sync.dma_start(
                    out=v_sb[:].rearrange("p (n d) -> p n d", d=128),
                    in_=v[u].rearrange("(n p) d -> p n d", p=128),
                )
                nc.sync.dma_start(out=qraw[:], in_=qT[u])
                nc.sync.dma_start(out=kraw[:], in_=kT[u])
                rq = bigpool.tile([128, s], dt.float32r, tag="rq")
                rk = bigpool.tile([128, s], dt.float32r, tag="rk")
                return qraw, kraw, v_sb, rq, rk

            def rotary(tiles, lo, hi):
                """Rotary for columns [lo,hi) of both q and k; all DVE ops in
                <=1024-col chunks so concurrent mask ops aren't starved."""
                qraw, kraw, v_sb, rq, rk = tiles
                for raw, r in ((qraw, rq), (kraw, rk)):
                    for c2 in range(lo // 1024, hi // 1024):
                        sl2 = slice(c2 * 1024, (c2 + 1) * 1024)
                        sw = ps_sc.tile([128, 1024], dt.float32, tag="sc")
                        for hh in range(2):
                            sl = slice(c2 * 1024 + hh * 512, c2 * 1024 + (hh + 1) * 512)
                            nc.tensor.matmul(
                                sw[:, hh * 512 : (hh + 1) * 512],
                                pswap_sb[:],
                                raw[:, sl],
                                start=True,
                                stop=True,
                            )
                        # r = raw * cexp
                        nc.vector.tensor_mul(r[:, sl2], raw[:, sl2], cexp_sb[:, sl2])
                        # raw <- swap(raw) * ssig  (psum src; raw reused as scratch)
                        nc.vector.tensor_mul(
                            raw[:, sl2], sw[:].bitcast(dt.float32r), ssig_sb[:, sl2]
                        )
                        # r += scratch
                        nc.vector.tensor_add(r[:, sl2], r[:, sl2], raw[:, sl2])

            def attention(u, rq, rk, v_sb, qbs=None):
                for qb in (range(nqb) if qbs is None else qbs):
                    outT_ps = ps_out.tile([128, QB], dt.float32, tag="outT")
                    den_ps = ps_den.tile([128, QB], dt.float32, tag="den")
                    kjs = kj_list(qb, s=s, w=w)
                    tiles = []
                    for kj in kjs:
                        kinds = chunk_kinds(qb, kj, w=w)
                        assert kinds, (qb, kj)
                        tiles.append((kj, kinds, kinds[0][0], kinds[-1][0] + 1))
                    # process full-width tiles first (kj=0 leads; pairs share
                    # one 2-bank psum tile + one exp), then partial tiles
                    fulls = [x for x in tiles if x[3] - x[2] == NQC]
                    parts = [x for x in tiles if x[3] - x[2] != NQC]
                    assert fulls[0][0] == 0
                    groups = []
                    i = 0
                    while i < len(fulls):
                        groups.append(fulls[i : i + 2])
                        i += 2
                    groups.extend([x] for x in parts)
                    n_groups = len(groups)

                    for gi, group in enumerate(groups):
                        lastg = gi == n_groups - 1
                        sc = ps_sc.tile([128, 2 * QB], dt.float32, tag="sc")
                        probs = ppool.tile([128, 2 * QB], dt.float32r, tag="probs")
                        for hh, (kj, kinds, t0, t1) in enumerate(group):
                            csl = slice(qb * QB + t0 * 128, qb * QB + t1 * 128)
                            hsl = slice(hh * QB + t0 * 128, hh * QB + t1 * 128)
                            ksl = slice(kj * 128, (kj + 1) * 128)
                            nc.tensor.matmul(
                                sc[:, hsl], rk[:, ksl], rq[:, csl],
                                start=True, stop=True,
                            )
                        lo = group[0][2] * 128
                        hi = (len(group) - 1) * QB + group[-1][3] * 128
                        nc.scalar.activation(
                            probs[:, lo:hi],
                            sc[:, lo:hi],
                            mybir.ActivationFunctionType.Exp,
                            scale=SCALE,
                        )
                        for hh, (kj, kinds, t0, t1) in enumerate(group):
                            # coalesce adjacent window masks into one op
                            mk = [k for k in kinds if k[1] != "full"]
                            j = 0
                            while j < len(mk):
                                t, kind = mk[j]
                                if (
                                    wpair_sb is not None
                                    and j + 1 < len(mk)
                                    and kind != "diag"
                                    and mk[j + 1][1] != "diag"
                                    and mk[j + 1][0] == t + 1
                                    and kind[1] == wdeltas[0]
                                ):
                                    tsl = slice(
                                        hh * QB + t * 128, hh * QB + (t + 2) * 128
                                    )
                                    nc.vector.tensor_mul(
                                        probs[:, tsl], probs[:, tsl], wpair_sb[:]
                                    )
                                    j += 2
                                    continue
                                m = (
                                    maskD_sb
                                    if kind == "diag"
                                    else wmask_sb[kind[1]]
                                )
                                tsl = slice(
                                    hh * QB + t * 128, hh * QB + (t + 1) * 128
                                )
                                nc.vector.tensor_mul(
                                    probs[:, tsl], probs[:, tsl], m[:]
                                )
                                j += 1
                            psl = slice(t0 * 128, t1 * 128)
                            hsl = slice(hh * QB + t0 * 128, hh * QB + t1 * 128)
                            ksl = slice(kj * 128, (kj + 1) * 128)
                            first = kj == 0
                            last = lastg and hh == len(group) - 1
                            nc.tensor.matmul(
                                outT_ps[:, psl],
                                v_sb[:, ksl],
                                probs[:, hsl],
                                start=first,
                                stop=last,
                            )
                            nc.tensor.matmul(
                                den_ps[:, psl],
                                ones_sb[:],
                                probs[:, hsl],
                                start=first,
                                stop=last,
                            )

                    # ---- tail: normalize, transpose, store ----
                    rden = tpool.tile([128, QB], dt.float32, tag="rden")
                    nc.vector.reciprocal_approx_fast(rden[:], den_ps[:])
                    outN = tpool.tile([128, QB], dt.float32, tag="outN")
                    nc.vector.tensor_mul(outN[:], outT_ps[:], rden[:])
                    tr = ps_tr.tile([128, QB], dt.float32, tag="tr")
                    for c in range(NQC):
                        tsl = slice(c * 128, (c + 1) * 128)
                        nc.tensor.transpose(tr[:, tsl], outN[:, tsl], ident_sb[:])
                    out_sb = tpool.tile([128, QB], dt.float32, tag="out_sb")
                    nc.scalar.copy(out_sb[:], tr[:])
                    nc.sync.dma_start(
                        out=out[u].rearrange("(n p) d -> p n d", p=128)[
                            :, qb * NQC : (qb + 1) * NQC, :
                        ],
                        in_=out_sb[:].rearrange("p (n d) -> p n d", d=128),
                    )

            split = 2048 if s >= 2048 else s
            t0 = load(0)
            rotary(t0, 0, split)
            pending = [t0]
            for u in range(per_core):
                cur = pending.pop(0)
                if u == 0 and split < s:
                    attention(u, cur[3], cur[4], cur[2], qbs=range(0, split // QB))
                    rotary(cur, split, s)
                    attention(u, cur[3], cur[4], cur[2], qbs=range(split // QB, nqb))
                else:
                    attention(u, cur[3], cur[4], cur[2])
                if u + 1 < per_core:
                    t = load(u + 1)
                    rotary(t, 0, s)
                    pending.append(t)

    nc.compile()
    return nc


def host_prep(q, k, v, cos, sin, s=S, w=LOCAL_WINDOW):
    """Build per-core input maps from full inputs."""
    b, _, h, d = q.shape
    partial = _window_partial_deltas(w)

    cexp = np.empty((128, s), dtype=np.float32)
    ssig = np.empty((128, s), dtype=np.float32)
    cexp[0::2, :] = cos.T
    cexp[1::2, :] = cos.T
    ssig[0::2, :] = -sin.T
    ssig[1::2, :] = sin.T

    pswap = np.zeros((128, 128), dtype=np.float32)
    idx = np.arange(128)
    pswap[idx, idx ^ 1] = 1.0
    ident = np.eye(128, dtype=np.float32)
    ones = np.ones((128, 128), dtype=np.float32)

    p = np.arange(128)[:, None]
    c = np.arange(128)[None, :]
    maskD = (c >= p).astype(np.float32)
    wmasks = {
        delta: ((c - p) < t).astype(np.float32) for delta, t in partial.items()
    }

    units = [(bi, hi) for bi in range(b) for hi in range(h)]
    per = len(units) // N_CORES
    in_maps = []
    for core in range(N_CORES):
        us = units[core * per : (core + 1) * per]
        qTc = np.ascontiguousarray(
            np.stack([q[bi, :, hi, :].T for bi, hi in us])
        )
        kTc = np.ascontiguousarray(
            np.stack([k[bi, :, hi, :].T for bi, hi in us])
        )
        vc = np.ascontiguousarray(np.stack([v[bi, :, hi, :] for bi, hi in us]))
        m = {
            "qT": qTc,
            "kT": kTc,
            "v": vc,
            "cexpT": cexp,
            "ssigT": ssig,
            "pswap": pswap,
            "ident": ident,
            "ones": ones,
            "maskD": maskD,
        }
        for delta, msk in wmasks.items():
            m[f"maskW{delta}"] = msk
        in_maps.append(m)
    return in_maps, units


_NC_CACHE = {}


def kernel(q, k, v, cos, sin):
    from concourse.bass_utils import run_bass_kernel_spmd

    q = np.asarray(q, dtype=np.float32)
    k = np.asarray(k, dtype=np.float32)
    v = np.asarray(v, dtype=np.float32)
    cos = np.asarray(cos, dtype=np.float32)
    sin = np.asarray(sin, dtype=np.float32)

    if "nc" not in _NC_CACHE:
        _NC_CACHE["nc"] = build_nc()
    nc = _NC_CACHE["nc"]

    in_maps, units = host_prep(q, k, v, cos, sin)
    res = run_bass_kernel_spmd(nc, in_maps, core_ids=list(range(N_CORES)))

    b, s, h, d = q.shape
    full = np.empty((b, s, h, d), dtype=np.float32)
    per = len(units) // N_CORES
    for core in range(N_CORES):
        o = res.results[core]["out"]  # [per, s, 128]
        for i, (bi, hi) in enumerate(units[core * per : (core + 1) * per]):
            full[bi, :, hi, :] = o[i]
    return full


# revision 14
# speedup vs baseline: 1.4282x; 1.2210x over previous
"""Trainium2 Bass kernel for BaseCausalWanSelfAttention (local+sink sparse attention
with interleaved rotary), SPMD across 8 NeuronCores.

Sharding: the 24 (batch, head) pairs are split 3-per-core across 8 cores; each
core runs full local+sink attention for its pairs independently (no collectives).
"""
import sys

sys.path.insert(0, "/opt/trn_rl_repo")

import numpy as np

import concourse.bacc as bacc
import concourse.tile as tile
import concourse.mybir as mybir

dt = mybir.dt

# Problem config (hardcoded per contest contract)
B, S, H, D = 2, 3072, 12, 128
LOCAL_WINDOW = 1560
SINK = 128
N_CORES = 8
PER_CORE = (B * H) // N_CORES  # 3
QB = 512  # query block (columns of transposed scores)
NQC = QB // 128  # 128-query chunks per block
SCALE = 1.0 / float(np.sqrt(D))


def _window_partial_deltas(w):
    """k-tile offsets (qi - kj) where the local-window edge cuts through the
    128x128 tile; maps delta -> threshold T with allowed iff (c - p) < T."""
    out = {}
    for d in range((w - 127 + 127) // 128, (w + 127) // 128 + 1):
        t = w - 128 * d
        if -127 <= t <= 127:
            out[d] = t
    return out


def chunk_kinds(qb, kj, w=LOCAL_WINDOW, nqc=NQC):
    """Per 128-query chunk classification of k-tile kj for query block qb.
    Returns list of (t, kind) with kind in {"full", "diag", ("win", delta)} for
    valid chunks only. SINK==128 assumed (k-tile 0 fully attendable)."""
    partial = _window_partial_deltas(w)
    max_delta = max(partial) if partial else (w - 1) // 128
    kinds = []
    for t in range(nqc):
        qi = nqc * qb + t
        if kj == 0:
            kinds.append((t, "diag" if qi == 0 else "full"))
            continue
        delta = qi - kj
        if delta < 0 or delta > max_delta:
            continue
        if delta == 0:
            kinds.append((t, "diag"))
        elif delta in partial:
            kinds.append((t, ("win", delta)))
        else:
            kinds.append((t, "full"))
    return kinds


def kj_list(qb, s=S, w=LOCAL_WINDOW, nqc=NQC):
    partial = _window_partial_deltas(w)
    max_delta = max(partial) if partial else (w - 1) // 128
    n_ktiles = s // 128
    hi = min(nqc * qb + nqc - 1, n_ktiles - 1)
    lo = max(1, nqc * qb - max_delta)
    return [0] + [kj for kj in range(lo, hi + 1)]


def build_nc(s=S, per_core=PER_CORE, w=LOCAL_WINDOW):
    """Build the SPMD single-core program (identical on all cores)."""
    nqb = s // QB
    partial = _window_partial_deltas(w)

    nc = bacc.Bacc("TRN2", target_bir_lowering=False, debug=False)

    qT = nc.declare_dram_parameter("qT", [per_core, 128, s], dt.float32r, isOutput=False)
    kT = nc.declare_dram_parameter("kT", [per_core, 128, s], dt.float32r, isOutput=False)
    v = nc.declare_dram_parameter("v", [per_core, s, 128], dt.float32r, isOutput=False)
    cexpT = nc.declare_dram_parameter("cexpT", [128, s], dt.float32r, isOutput=False)
    ssigT = nc.declare_dram_parameter("ssigT", [128, s], dt.float32r, isOutput=False)
    pswap = nc.declare_dram_parameter("pswap", [128, 128], dt.float32r, isOutput=False)
    ident = nc.declare_dram_parameter("ident", [128, 128], dt.float32, isOutput=False)
    ones = nc.declare_dram_parameter("ones", [128, 128], dt.float32r, isOutput=False)
    maskD = nc.declare_dram_parameter("maskD", [128, 128], dt.float32r, isOutput=False)
    wmask_names = {}
    for delta in sorted(partial):
        nm = f"maskW{delta}"
        wmask_names[delta] = nc.declare_dram_parameter(
            nm, [128, 128], dt.float32r, isOutput=False
        )
    out = nc.declare_dram_parameter("out", [per_core, s, 128], dt.float32, isOutput=True)

    with tile.TileContext(nc) as tc:
        with (
            tc.tile_pool(name="const", bufs=1) as cpool,
            tc.tile_pool(name="big", bufs=2) as bigpool,
            tc.tile_pool(name="probs", bufs=8) as ppool,
            tc.tile_pool(name="tail", bufs=2) as tpool,
            tc.tile_pool(name="ps_sc", bufs=4, space="PSUM") as ps_sc,
            tc.tile_pool(name="ps_out", bufs=2, space="PSUM") as ps_out,
            tc.tile_pool(name="ps_den", bufs=1, space="PSUM") as ps_den,
            tc.tile_pool(name="ps_tr", bufs=1, space="PSUM") as ps_tr,
        ):
            # constants
            cexp_sb = cpool.tile([128, s], dt.float32r, tag="cexp")
            ssig_sb = cpool.tile([128, s], dt.float32r, tag="ssig")
            for c2 in range(s // 1024):
                sl2 = slice(c2 * 1024, (c2 + 1) * 1024)
                nc.sync.dma_start(out=cexp_sb[:, sl2], in_=cexpT[:, sl2])
                nc.sync.dma_start(out=ssig_sb[:, sl2], in_=ssigT[:, sl2])
            pswap_sb = cpool.tile([128, 128], dt.float32r, tag="pswap")
            ident_sb = cpool.tile([128, 128], dt.float32, tag="ident")
            ones_sb = cpool.tile([128, 128], dt.float32r, tag="ones")
            nc.sync.dma_start(out=pswap_sb[:], in_=pswap[:])
            nc.sync.dma_start(out=ident_sb[:], in_=ident[:])
            nc.sync.dma_start(out=ones_sb[:], in_=ones[:])
            maskD_sb = cpool.tile([128, 128], dt.float32r, tag="maskD")
            nc.sync.dma_start(out=maskD_sb[:], in_=maskD[:])
            wmask_sb = {}
            wdeltas = sorted(wmask_names)
            for delta, ap in wmask_names.items():
                m = cpool.tile([128, 128], dt.float32r, tag=f"maskW{delta}")
                nc.sync.dma_start(out=m[:], in_=ap[:])
                wmask_sb[delta] = m
            # combined [W_d | W_{d+1}] tile for adjacent window chunks
            wpair_sb = None
            if len(wdeltas) == 2 and wdeltas[1] == wdeltas[0] + 1:
                wpair_sb = cpool.tile([128, 256], dt.float32r, tag="maskWpair")
                nc.sync.dma_start(
                    out=wpair_sb[:, 0:128], in_=wmask_names[wdeltas[0]][:]
                )
                nc.sync.dma_start(
                    out=wpair_sb[:, 128:256], in_=wmask_names[wdeltas[1]][:]
                )

            def load(u):
                qraw = bigpool.tile([128, s], dt.float32r, tag="qraw", bufs=1)
                kraw = bigpool.tile([128, s], dt.float32r, tag="kraw", bufs=1)
                v_sb = bigpool.tile([128, s], dt.float32r, tag="v")
                nc.sync.dma_start(
                    out=v_sb[:].rearrange("p (n d) -> p n d", d=128),
                    in_=v[u].rearrange("(n p) d -> p n d", p=128),
                )
                for c2 in range(s // 1024):
                    sl2 = slice(c2 * 1024, (c2 + 1) * 1024)
                    nc.sync.dma_start(out=qraw[:, sl2], in_=qT[u][:, sl2])
                    nc.sync.dma_start(out=kraw[:, sl2], in_=kT[u][:, sl2])
                rq = bigpool.tile([128, s], dt.float32r, tag="rq")
                rk = bigpool.tile([128, s], dt.float32r, tag="rk")
                return qraw, kraw, v_sb, rq, rk

            def rotary(tiles, lo, hi):
                """Rotary for columns [lo,hi) of both q and k; all DVE ops in
                <=1024-col chunks so concurrent mask ops aren't starved."""
                qraw, kraw, v_sb, rq, rk = tiles
                for raw, r in ((qraw, rq), (kraw, rk)):
                    for c2 in range(lo // 1024, hi // 1024):
                        sl2 = slice(c2 * 1024, (c2 + 1) * 1024)
                        sws = []
                        for hh in range(2):
                            sl = slice(c2 * 1024 + hh * 512, c2 * 1024 + (hh + 1) * 512)
                            sw = ps_sc.tile([128, 512], dt.float32, tag="sc")
                            nc.tensor.matmul(
                                sw[:], pswap_sb[:], raw[:, sl], start=True, stop=True
                            )
                            sws.append((sl, sw))
                        # r = raw * cexp
                        nc.vector.tensor_mul(r[:, sl2], raw[:, sl2], cexp_sb[:, sl2])
                        # raw <- swap(raw) * ssig  (psum src; raw reused as scratch)
                        for sl, sw in sws:
                            nc.vector.tensor_mul(
                                raw[:, sl], sw[:].bitcast(dt.float32r), ssig_sb[:, sl]
                            )
                        # r += scratch
                        nc.vector.tensor_add(r[:, sl2], r[:, sl2], raw[:, sl2])

            def attention(u, rq, rk, v_sb, qbs=None):
                for qb in (range(nqb) if qbs is None else qbs):
                    outT_ps = ps_out.tile([128, QB], dt.float32, tag="outT")
                    den_ps = ps_den.tile([128, QB], dt.float32, tag="den")
                    kjs = kj_list(qb, s=s, w=w)
                    tiles = []
                    for kj in kjs:
                        kinds = chunk_kinds(qb, kj, w=w)
                        assert kinds, (qb, kj)
                        tiles.append((kj, kinds, kinds[0][0], kinds[-1][0] + 1))
                    # full-width tiles first (kj=0 leads), then partial tiles
                    fulls = [x for x in tiles if x[3] - x[2] == NQC]
                    parts = [x for x in tiles if x[3] - x[2] != NQC]
                    assert fulls[0][0] == 0
                    order = fulls + parts
                    n_tiles = len(order)

                    for ti, (kj, kinds, t0, t1) in enumerate(order):
                        csl = slice(qb * QB + t0 * 128, qb * QB + t1 * 128)
                        psl = slice(t0 * 128, t1 * 128)
                        ksl = slice(kj * 128, (kj + 1) * 128)
                        first = kj == 0
                        last = ti == n_tiles - 1

                        sc = ps_sc.tile([128, QB], dt.float32, tag="sc")
                        nc.tensor.matmul(
                            sc[:, psl], rk[:, ksl], rq[:, csl], start=True, stop=True
                        )
                        probs = ppool.tile([128, QB], dt.float32r, tag="probs")
                        nc.scalar.activation(
                            probs[:, psl],
                            sc[:, psl],
                            mybir.ActivationFunctionType.Exp,
                            scale=SCALE,
                        )
                        # masks: coalesce adjacent window chunks into one op
                        mk = [k for k in kinds if k[1] != "full"]
                        j = 0
                        while j < len(mk):
                            t, kind = mk[j]
                            if (
                                wpair_sb is not None
                                and j + 1 < len(mk)
                                and kind != "diag"
                                and mk[j + 1][1] != "diag"
                                and mk[j + 1][0] == t + 1
                                and kind[1] == wdeltas[0]
                            ):
                                tsl = slice(t * 128, (t + 2) * 128)
                                nc.vector.tensor_mul(
                                    probs[:, tsl], probs[:, tsl], wpair_sb[:]
                                )
                                j += 2
                                continue
                            m = maskD_sb if kind == "diag" else wmask_sb[kind[1]]
                            tsl = slice(t * 128, (t + 1) * 128)
                            nc.vector.tensor_mul(probs[:, tsl], probs[:, tsl], m[:])
                            j += 1
                        nc.tensor.matmul(
                            outT_ps[:, psl],
                            v_sb[:, ksl],
                            probs[:, psl],
                            start=first,
                            stop=last,
                        )
                        nc.tensor.matmul(
                            den_ps[:, psl],
                            ones_sb[:],
                            probs[:, psl],
                            start=first,
                            stop=last,
                        )

                    # ---- tail: normalize, transpose, store ----
                    rden = tpool.tile([128, QB], dt.float32, tag="rden")
                    nc.vector.reciprocal_approx_fast(rden[:], den_ps[:])
                    outN = tpool.tile([128, QB], dt.float32, tag="outN")
                    nc.vector.tensor_mul(outN[:], outT_ps[:], rden[:])
                    tr = ps_tr.tile([128, QB], dt.float32, tag="tr")
                    for c in range(NQC):
                        tsl = slice(c * 128, (c + 1) * 128)
                        nc.tensor.transpose(tr[:, tsl], outN[:, tsl], ident_sb[:])
                    out_sb = tpool.tile([128, QB], dt.float32, tag="out_sb")
                    nc.scalar.copy(out_sb[:], tr[:])
                    nc.sync.dma_start(
                        out=out[u].rearrange("(n p) d -> p n d", p=128)[
                            :, qb * NQC : (qb + 1) * NQC, :
                        ],
                        in_=out_sb[:].rearrange("p (n d) -> p n d", d=128),
                    )

            split = 2048 if s >= 2048 else s
            t0 = load(0)
            rotary(t0, 0, split)
            pending = [t0]
            for u in range(per_core):
                cur = pending.pop(0)
                if u == 0 and split < s:
                    attention(u, cur[3], cur[4], cur[2], qbs=range(0, split // QB))
                    rotary(cur, split, s)
                    attention(u, cur[3], cur[4], cur[2], qbs=range(split // QB, nqb))
                else:
                    attention(u, cur[3], cur[4], cur[2])
                if u + 1 < per_core:
                    t = load(u + 1)
                    rotary(t, 0, s)
                    pending.append(t)

    nc.compile()
    return nc


def host_prep(q, k, v, cos, sin, s=S, w=LOCAL_WINDOW):
    """Build per-core input maps from full inputs."""
    b, _, h, d = q.shape
    partial = _window_partial_deltas(w)

    cexp = np.empty((128, s), dtype=np.float32)
    ssig = np.empty((128, s), dtype=np.float32)
    cexp[0::2, :] = cos.T
    cexp[1::2, :] = cos.T
    ssig[0::2, :] = -sin.T
    ssig[1::2, :] = sin.T

    pswap = np.zeros((128, 128), dtype=np.float32)
    idx = np.arange(128)
    pswap[idx, idx ^ 1] = 1.0
    ident = np.eye(128, dtype=np.float32)
    ones = np.ones((128, 128), dtype=np.float32)

    p = np.arange(128)[:, None]
    c = np.arange(128)[None, :]
    maskD = (c >= p).astype(np.float32)
    wmasks = {
        delta: ((c - p) < t).astype(np.float32) for delta, t in partial.items()
    }

    units = [(bi, hi) for bi in range(b) for hi in range(h)]
    per = len(units) // N_CORES
    in_maps = []
    for core in range(N_CORES):
        us = units[core * per : (core + 1) * per]
        qTc = np.ascontiguousarray(
            np.stack([q[bi, :, hi, :].T for bi, hi in us])
        )
        kTc = np.ascontiguousarray(
            np.stack([k[bi, :, hi, :].T for bi, hi in us])
        )
        vc = np.ascontiguousarray(np.stack([v[bi, :, hi, :] for bi, hi in us]))
        m = {
            "qT": qTc,
            "kT": kTc,
            "v": vc,
            "cexpT": cexp,
            "ssigT": ssig,
            "pswap": pswap,
            "ident": ident,
            "ones": ones,
            "maskD": maskD,
        }
        for delta, msk in wmasks.items():
            m[f"maskW{delta}"] = msk
        in_maps.append(m)
    return in_maps, units


_NC_CACHE = {}


def kernel(q, k, v, cos, sin):
    from concourse.bass_utils import run_bass_kernel_spmd

    q = np.asarray(q, dtype=np.float32)
    k = np.asarray(k, dtype=np.float32)
    v = np.asarray(v, dtype=np.float32)
    cos = np.asarray(cos, dtype=np.float32)
    sin = np.asarray(sin, dtype=np.float32)

    if "nc" not in _NC_CACHE:
        _NC_CACHE["nc"] = build_nc()
    nc = _NC_CACHE["nc"]

    in_maps, units = host_prep(q, k, v, cos, sin)
    res = run_bass_kernel_spmd(nc, in_maps, core_ids=list(range(N_CORES)))

    b, s, h, d = q.shape
    full = np.empty((b, s, h, d), dtype=np.float32)
    per = len(units) // N_CORES
    for core in range(N_CORES):
        o = res.results[core]["out"]  # [per, s, 128]
        for i, (bi, hi) in enumerate(units[core * per : (core + 1) * per]):
            full[bi, :, hi, :] = o[i]
    return full
